# revision 42
# baseline (speedup 1.0000x reference)
"""Multi-head self-attention (B=8, T=2048, C=192, H=6, HS=32) on 8 TRN2 cores.

Sharding: data-parallel over batch - core i computes batch element i fully
on-chip (no collectives). Host pre-transposes x and packs weights.

Per core:
  qkT [384, t]  = wqk.T @ x.T, packed into 3x[128, T] tiles (q0-3 | q4,q5,k0,k1 | k2-5)
  v_aug [s, 6*33] = x @ Wv, per-head [v_h | ones] (33 cols)
  S^T [s, t]    = kT_h.T @ qT_h        (per (si, h), one 512-wide matmul)
  P^T           = exp(S^T / sqrt(HS))  split across 3 engines:
                    ScalarE: exact exp activation
                    DVE/GPSIMD: Schraudolph exp via int16 bit trick
                      bf16bits(exp(s)) ~ int16(s*AEXP + BEXP)
  O[t, d+sum]   = PV with P^T as STATIONARY (cost ~ 33/row vs 512/row)
  normalize per-partition 1/rowsum (tensor_scalar with per-partition scalar)
  O^T via PE transpose; Y[t, c] = O^T.T @ Wproj + bias; DMA out.
"""

import numpy as np
import ml_dtypes
from collections import deque
from contextlib import ExitStack

import concourse.bass as bass
import concourse.tile as tile
from concourse import bacc, mybir
from concourse.bass_utils import run_bass_kernel_spmd

B, T, C = 8, 2048, 192
H, HS = 6, 32
P = 128
TCH = 512            # t-chunk width (one PSUM bank of fp32)
NCH = T // TCH       # 4
NS = T // P          # 16 s-tiles
NJ = TCH // P        # 4 t-tiles per chunk
SCALE = 1.0 / float(np.sqrt(HS))
BF16 = mybir.dt.bfloat16
F32 = mybir.dt.float32
I16 = mybir.dt.int16
Exp = mybir.ActivationFunctionType.Exp
MUL = mybir.AluOpType.mult
ADD = mybir.AluOpType.add

# Schraudolph exp in bf16-bit domain: bf16_bits = int16(s * AEXP + BEXP)
AEXP = float(SCALE * 128.0 * np.log2(np.e))
BEXP = 16248.5

# per-chunk engine split for the 48 exp pair-tiles (GPSIMD cannot access
# PSUM, so only ScalarE (exact exp) and DVE (Schraudolph) share the work)
W_SC, W_DV = 27, 21


def _mk_pattern():
    ev = sorted(
        [((i + 0.5) / W_SC, 0) for i in range(W_SC)]
        + [((i + 0.5) / W_DV, 1) for i in range(W_DV)]
    )
    return [e for _, e in ev]


EXP_PAT = _mk_pattern()
LAG = 3  # software pipeline depth (in pair-steps) between QKT and PV

_CACHE = {}


def build_nc():
    nc = bacc.Bacc()
    xT = nc.declare_dram_parameter("xT", [C, T], BF16, isOutput=False)
    wqk = nc.declare_dram_parameter("wqk", [C, 2 * H * HS], BF16, isOutput=False)
    wv = nc.declare_dram_parameter("wv", [C, H * HS], BF16, isOutput=False)
    wp = nc.declare_dram_parameter("wp", [H * HS, C], BF16, isOutput=False)
    bp = nc.declare_dram_parameter("bp", [1, C], BF16, isOutput=False)
    ident = nc.declare_dram_parameter("ident", [P, P], BF16, isOutput=False)
    out = nc.declare_dram_parameter("out", [T, C], F32, isOutput=True)

    with tile.TileContext(nc) as tc, ExitStack() as ctx:
        singles = ctx.enter_context(tc.tile_pool(name="singles", bufs=1))
        qk_pool = ctx.enter_context(tc.tile_pool(name="qk", bufs=1))
        va_pool = ctx.enter_context(tc.tile_pool(name="va", bufs=1))
        pt_pool = ctx.enter_context(tc.tile_pool(name="pt", bufs=1))
        on_pool = ctx.enter_context(tc.tile_pool(name="on", bufs=3))
        ot_pool = ctx.enter_context(tc.tile_pool(name="ot", bufs=4))
        rc_pool = ctx.enter_context(tc.tile_pool(name="rc", bufs=2))
        ysb_pool = ctx.enter_context(tc.tile_pool(name="ysb", bufs=3))
        ps_pool = ctx.enter_context(tc.tile_pool(name="ps", bufs=3, space="PSUM"))
        po_pool = ctx.enter_context(tc.tile_pool(name="po", bufs=2, space="PSUM"))

        def ps_tile(name):
            return ps_pool.tile([P, 2 * TCH], F32, name=name, tag="ps")

        # ---------------- load inputs ----------------
        # weights + first x column-chunk first, so phase 1/2 start early
        wqk_a = singles.tile([P, 2 * H * HS], BF16)
        nc.sync.dma_start(wqk_a, wqk[0:P, :])
        wqk_b = singles.tile([C - P, 2 * H * HS], BF16)
        nc.sync.dma_start(wqk_b, wqk[P:C, :])
        xT_a = singles.tile([P, T], BF16)
        nc.sync.dma_start(xT_a[:, 0:TCH], xT[0:P, 0:TCH])
        xT_b = singles.tile([C - P, T], BF16)
        nc.sync.dma_start(xT_b[:, 0:TCH], xT[P:C, 0:TCH])
        wv_a = singles.tile([P, H * HS], BF16)
        nc.sync.dma_start(wv_a, wv[0:P, :])
        wv_b = singles.tile([C - P, H * HS], BF16)
        nc.sync.dma_start(wv_b, wv[P:C, :])
        nc.sync.dma_start(xT_a[:, TCH:T], xT[0:P, TCH:T])
        nc.sync.dma_start(xT_b[:, TCH:T], xT[P:C, TCH:T])
        wp_a = singles.tile([P, C], BF16)
        nc.sync.dma_start(wp_a, wp[0:P, :])
        # second k-slab of Wproj lives at partition base 64 so it matches the
        # base of the overlapping DMA-transpose output OTy (d rows 64..191);
        # a base-0 copy (wp_b) serves the last chunk's PE-transpose path.
        wp_bb = singles.tile([P, C], BF16)
        nc.sync.dma_start(wp_bb[64:P, :], wp[P:H * HS, :])
        wp_b = singles.tile([H * HS - P, C], BF16)
        nc.sync.dma_start(wp_b, wp[P:H * HS, :])
        bp_sb = singles.tile([1, C], BF16)
        nc.sync.dma_start(bp_sb, bp[:, :])
        ident_sb = singles.tile([P, P], BF16)
        nc.sync.dma_start(ident_sb, ident[:, :])
        ones1 = singles.tile([1, P], BF16)
        nc.vector.memset(ones1, 1.0)

        # ---------------- phase 1 producers (emitted lazily) ----------------
        # PE requires fmap/weight at the same partition base, so q_h and k_h
        # live in separate tiles at identical row offsets.
        # wqk col order: [q0-3 (128) | k0-3 (128) | q4,q5 (64) | k4,k5 (64)]
        qkt = [singles.tile([P, T], BF16, name="qA"),
               singles.tile([P, T], BF16, name="kA"),
               singles.tile([P - 64, T], BF16, name="qB"),
               singles.tile([P - 64, T], BF16, name="kB")]
        qk_cols = [(0, P), (P, P), (2 * P, 64), (2 * P + 64, 64)]
        cp_engines = [nc.scalar.copy, nc.vector.tensor_copy]
        ci = [0]

        def emit_qk(g, c):
            c0, csz = qk_cols[g]
            ps = ps_tile("psqk")
            nc.tensor.matmul(ps[0:csz, 0:TCH], wqk_a[:, c0:c0 + csz],
                             xT_a[:, c * TCH:(c + 1) * TCH],
                             start=True, stop=False)
            nc.tensor.matmul(ps[0:csz, 0:TCH], wqk_b[:, c0:c0 + csz],
                             xT_b[:, c * TCH:(c + 1) * TCH],
                             start=False, stop=True)
            cp_engines[ci[0] % 2](qkt[g][:, c * TCH:(c + 1) * TCH],
                                  ps[0:csz, 0:TCH])
            ci[0] += 1

        va = [va_pool.tile([P, H * 33], BF16, name=f"va{si}")
              for si in range(NS)]

        def emit_va(si):
            ps = ps_tile("psv")
            nc.tensor.matmul(ps[:, 0:H * HS], xT_a[:, si * P:(si + 1) * P],
                             wv_a, start=True, stop=False)
            nc.tensor.matmul(ps[:, 0:H * HS], xT_b[:, si * P:(si + 1) * P],
                             wv_b, start=False, stop=True)
            va_r = va[si].rearrange("p (h e) -> p h e", h=H)
            ps_r = ps[:, 0:H * HS].rearrange("p (h d) -> p h d", h=H)
            cp_engines[ci[0] % 2](va_r[:, :, 0:HS], ps_r)
            ci[0] += 1
            nc.gpsimd.memset(va_r[:, :, 32], 1.0)

        # head -> (tile, row) maps; q_h and k_h share the same row base
        def qsrc(h):
            return (qkt[0], HS * h) if h < 4 else (qkt[2], HS * (h - 4))

        def ksrc(h):
            return (qkt[1], HS * h) if h < 4 else (qkt[3], HS * (h - 4))

        # chunk-0 interleave schedule: emit each producer just before its
        # first phase-2 consumer so the exp pipeline starts almost at t=0.
        sched = {}

        def sched_add(pr, fn, *args):
            sched.setdefault(pr, []).append((fn, args))

        sched_add(1, emit_qk, 1, 1)   # kA cols for si 4..7
        sched_add(5, emit_qk, 1, 2)   # si 8..11
        sched_add(9, emit_qk, 1, 3)   # si 12..15
        for si in range(1, NS):
            sched_add(si - 1, emit_va, si)
        sched_add(16, emit_qk, 2, 0)  # qB chunk 0 (heads 4,5 at pair 32)
        sched_add(18, emit_qk, 3, 0)  # kB cols for si 0..3
        sched_add(20, emit_qk, 3, 1)
        sched_add(22, emit_qk, 3, 2)
        sched_add(24, emit_qk, 3, 3)
        for i, (g, c) in enumerate(
                [(0, 1), (0, 2), (0, 3), (2, 1), (2, 2), (2, 3)]):
            sched_add(26 + 2 * i, emit_qk, g, c)  # later chunks' q columns

        # ---------------- phase 2 ----------------
        # ptp[p3][si]: P^T for head pair (2*p3, 2*p3+1); halves 512 cols each
        ptp = [[pt_pool.tile([P, 2 * TCH], BF16, name=f"pt{p3}_{si}")
                for si in range(NS)] for p3 in range(3)]

        # pre-seed: first QKT needs qA/kA chunk-0 columns and va[0]
        emit_qk(0, 0)
        emit_qk(1, 0)
        emit_va(0)

        def make_stagec(c, po2, last):
            """Stage C for chunk c as 3 task groups (emitted early in the
            NEXT chunk so the transpose latency hides under its QKTs)."""
            ons = []

            def norm_t(j):
                base = (j % 2) * 256
                po_r = po2[j // 2][:, base:base + H * 33].rearrange(
                    "p (h e) -> p h e", h=H)
                rcp = rc_pool.tile([P, 8], F32, name="rcp", tag="rc")
                nc.vector.reciprocal(rcp[:, 0:H], po_r[:, :, 32])
                on = on_pool.tile([P, C], BF16, name="on", tag="on")
                on_r = on[:, :].rearrange("p (h e) -> p h e", h=H)
                nc.vector.tensor_tensor(
                    on_r, po_r[:, :, 0:HS],
                    rcp[:, 0:H].to_broadcast([P, H, HS]), MUL)
                if not last:
                    # O^T via XBAR DMA transpose (SBUF->SBUF): d rows 0..127
                    # and (overlapping) 64..191, so both proj matmuls have
                    # matching fmap/weight partition bases (0 and 64).
                    otx = ot_pool.tile([P, P], BF16, name="otx", tag="otx")
                    oty = ot_pool.tile([P, P], BF16, name="oty", tag="oty")
                    nc.sync.dma_start_transpose(otx, on[:, 0:P])
                    nc.sync.dma_start_transpose(oty, on[:, 64:64 + P])
                else:
                    # tail chunk: PE transpose + engine copy (low latency)
                    psta = ps_tile("psta")
                    pstb = ps_tile("pstb")
                    nc.tensor.transpose(psta[:, 0:64].bitcast(BF16),
                                        on[:, 0:P], ident_sb)
                    nc.tensor.transpose(pstb[0:C - P, 0:64].bitcast(BF16),
                                        on[:, P:C], ident_sb)
                    otx = ot_pool.tile([P, P], BF16, name="otx", tag="otx")
                    oty = ot_pool.tile([C - P, P], BF16, name="otb",
                                       tag="otb")
                    nc.vector.tensor_copy(otx, psta[:, 0:64].bitcast(BF16))
                    nc.scalar.copy(oty, pstb[0:C - P, 0:64].bitcast(BF16))
                ons.append((otx, oty))

            def proj_t():
                psy2 = [po_pool.tile([P, TCH], F32, name=f"psy{jj}",
                                     tag=f"po{jj}", bufs=1)
                        for jj in range(2)]
                for jj in range(2):
                    for j in (2 * jj, 2 * jj + 1):
                        otx, oty = ons[j]
                        base = (j % 2) * 256
                        psy = psy2[jj][:, base:base + C]
                        nc.tensor.matmul(psy, otx, wp_a,
                                         start=(j % 2 == 0), stop=False)
                        if not last:
                            nc.tensor.matmul(
                                psy, oty[64:P, :], wp_bb[64:P, :],
                                start=False, stop=False,
                                tile_position=(64, 0),
                                skip_group_check=True)
                        else:
                            nc.tensor.matmul(psy, oty, wp_b,
                                             start=False, stop=False,
                                             tile_position=(0, 0),
                                             skip_group_check=True)
                        nc.tensor.matmul(psy, ones1, bp_sb,
                                         start=False, stop=(j % 2 == 1))
                    for j in (2 * jj, 2 * jj + 1):
                        t0 = c * TCH + j * P
                        base = (j % 2) * 256
                        ysb = ysb_pool.tile([P, C], F32, name="ysb", tag="y")
                        nc.vector.tensor_copy(ysb,
                                              psy2[jj][:, base:base + C])
                        nc.sync.dma_start(out[t0:t0 + P, :], ysb)

            return [lambda: (norm_t(0), norm_t(1)),
                    lambda: (norm_t(2), norm_t(3)),
                    proj_t]

        pending_stagec = []
        for c in range(NCH):
            pairs = [(si, p3) for p3 in range(3) for si in range(NS)]
            npair = len(pairs)
            po2box = []

            def issue_pv(pr):
                po2 = po2box[0]
                si, p3 = pairs[pr]
                for half in (0, 1):
                    h = 2 * p3 + half
                    for j in range(NJ):
                        base = (j % 2) * 256
                        nc.tensor.matmul(
                            po2[j // 2][:, base + h * 33:base + (h + 1) * 33],
                            ptp[p3][si][:, half * TCH + j * P:
                                        half * TCH + (j + 1) * P],
                            va[si][:, h * 33:(h + 1) * 33],
                            start=(pr == 0 and half == 0 and j % 2 == 0),
                            stop=(pr == npair - 1 and half == 1
                                  and j % 2 == 1),
                            tile_position=(0, 0), skip_group_check=True)

            for pr, (si, p3) in enumerate(pairs):
                if c == 0:
                    for fn, args in sched.get(pr, []):
                        fn(*args)
                if pending_stagec and pr < len(pending_stagec):
                    pending_stagec[pr]()
                if pr == LAG - 1:
                    # the previous chunk's stage C is fully traced; now the
                    # PSUM bank tags can rotate to this chunk's accumulators
                    pending_stagec = []
                    po2box.append(
                        [po_pool.tile([P, TCH], F32, name=f"po{jj}",
                                      tag=f"po{jj}", bufs=1)
                         for jj in range(2)])
                ps = ps_tile("psst")
                for half in (0, 1):
                    h = 2 * p3 + half
                    kt, kr = ksrc(h)
                    qt, qr = qsrc(h)
                    nc.tensor.matmul(
                        ps[:, half * TCH:(half + 1) * TCH],
                        kt[kr:kr + HS, si * P:(si + 1) * P],
                        qt[qr:qr + HS, c * TCH:(c + 1) * TCH],
                        start=True, stop=True,
                        tile_position=(kr, 0), skip_group_check=True)
                eng = EXP_PAT[pr]
                dst = ptp[p3][si]
                if eng == 0:
                    nc.scalar.activation(dst, ps, Exp, scale=SCALE)
                else:
                    nc.vector.tensor_scalar(dst[:, :].bitcast(I16), ps,
                                            AEXP, BEXP, MUL, ADD)
                if pr >= LAG:
                    issue_pv(pr - LAG)
            for pr in range(npair - LAG, npair):
                issue_pv(pr)

            last = (c == NCH - 1)
            tasks = make_stagec(c, po2box[0], last)
            if last:
                for t in tasks:
                    t()
            else:
                pending_stagec = tasks

    nc.compile()
    return nc


def _get_nc():
    if "nc" not in _CACHE:
        _CACHE["nc"] = build_nc()
    return _CACHE["nc"]


def make_in_maps(x, Wq, Wk, Wv, Wproj, bproj):
    bf = ml_dtypes.bfloat16
    x = np.asarray(x, np.float32)
    pack = lambda w: np.ascontiguousarray(
        np.transpose(np.asarray(w, np.float32), (1, 0, 2)).reshape(C, H * HS))
    pq, pk = pack(Wq), pack(Wk)
    # col order: q0-3 | k0-3 | q4,q5 | k4,k5 (q_h/k_h at equal row bases)
    wqk = np.ascontiguousarray(np.concatenate(
        [pq[:, 0:128], pk[:, 0:128], pq[:, 128:192], pk[:, 128:192]],
        axis=1)).astype(bf)
    wv = pack(Wv).astype(bf)
    wp = np.ascontiguousarray(np.asarray(Wproj, np.float32)).astype(bf)
    bp = np.asarray(bproj, np.float32).reshape(1, C).astype(bf)
    ident = np.eye(P, dtype=np.float32).astype(bf)
    maps = []
    for i in range(B):
        xti = np.ascontiguousarray(x[i].T).astype(bf)
        maps.append({"xT": xti, "wqk": wqk, "wv": wv, "wp": wp, "bp": bp,
                     "ident": ident})
    return maps


def run(inputs, trace=False, **kw):
    nc = _get_nc()
    in_maps = make_in_maps(**inputs)
    res = run_bass_kernel_spmd(nc, in_maps, core_ids=list(range(B)),
                               trace=trace, **kw)
    y = np.stack([np.asarray(res.results[i]["out"], np.float32)
                  for i in range(B)], axis=0)
    return y, res


def kernel(**inputs):
    y, _ = run(inputs, trace=False)
    return y


# revision 44
# speedup vs baseline: 1.0060x; 1.0060x over previous
"""Multi-head self-attention (B=8, T=2048, C=192, H=6, HS=32) on 8 TRN2 cores.

Sharding: data-parallel over batch - core i computes batch element i fully
on-chip (no collectives). Host pre-transposes x and packs weights.

Per core:
  qkT [384, t]  = wqk.T @ x.T, packed into 3x[128, T] tiles (q0-3 | q4,q5,k0,k1 | k2-5)
  v_aug [s, 6*33] = x @ Wv, per-head [v_h | ones] (33 cols)
  S^T [s, t]    = kT_h.T @ qT_h        (per (si, h), one 512-wide matmul)
  P^T           = exp(S^T / sqrt(HS))  split across 3 engines:
                    ScalarE: exact exp activation
                    DVE/GPSIMD: Schraudolph exp via int16 bit trick
                      bf16bits(exp(s)) ~ int16(s*AEXP + BEXP)
  O[t, d+sum]   = PV with P^T as STATIONARY (cost ~ 33/row vs 512/row)
  normalize per-partition 1/rowsum (tensor_scalar with per-partition scalar)
  O^T via PE transpose; Y[t, c] = O^T.T @ Wproj + bias; DMA out.
"""

import numpy as np
import ml_dtypes
from collections import deque
from contextlib import ExitStack

import concourse.bass as bass
import concourse.tile as tile
from concourse import bacc, mybir
from concourse.bass_utils import run_bass_kernel_spmd

B, T, C = 8, 2048, 192
H, HS = 6, 32
P = 128
TCH = 512            # t-chunk width (one PSUM bank of fp32)
NCH = T // TCH       # 4
NS = T // P          # 16 s-tiles
NJ = TCH // P        # 4 t-tiles per chunk
SCALE = 1.0 / float(np.sqrt(HS))
BF16 = mybir.dt.bfloat16
F32 = mybir.dt.float32
I16 = mybir.dt.int16
Exp = mybir.ActivationFunctionType.Exp
MUL = mybir.AluOpType.mult
ADD = mybir.AluOpType.add

# Schraudolph exp in bf16-bit domain: bf16_bits = int16(s * AEXP + BEXP)
AEXP = float(SCALE * 128.0 * np.log2(np.e))
BEXP = 16248.5

# per-chunk engine split for the 48 exp pair-tiles (GPSIMD cannot access
# PSUM, so only ScalarE (exact exp) and DVE (Schraudolph) share the work).
# The first 10 pairs are scalar-heavy: DVE digests the previous chunk's
# stage-C burst (recip/normalize/output copies) there.
def _mk_pattern():
    head = [0, 0, 0, 1, 0, 0, 0, 1, 0, 0]  # 8 scalar : 2 dve
    rest = sorted([((i + 0.5) / 19, 0) for i in range(19)]
                  + [((i + 0.5) / 19, 1) for i in range(19)])
    return head + [e for _, e in rest]      # totals: 27 scalar, 21 dve


EXP_PAT = _mk_pattern()
LAG = 3  # software pipeline depth (in pair-steps) between QKT and PV

_CACHE = {}


def build_nc():
    nc = bacc.Bacc()
    xT = nc.declare_dram_parameter("xT", [C, T], BF16, isOutput=False)
    wqk = nc.declare_dram_parameter("wqk", [C, 2 * H * HS], BF16, isOutput=False)
    wv = nc.declare_dram_parameter("wv", [C, H * HS], BF16, isOutput=False)
    wp = nc.declare_dram_parameter("wp", [H * HS, C], BF16, isOutput=False)
    bp = nc.declare_dram_parameter("bp", [1, C], BF16, isOutput=False)
    ident = nc.declare_dram_parameter("ident", [P, P], BF16, isOutput=False)
    out = nc.declare_dram_parameter("out", [T, C], F32, isOutput=True)

    with tile.TileContext(nc) as tc, ExitStack() as ctx:
        singles = ctx.enter_context(tc.tile_pool(name="singles", bufs=1))
        qk_pool = ctx.enter_context(tc.tile_pool(name="qk", bufs=1))
        va_pool = ctx.enter_context(tc.tile_pool(name="va", bufs=1))
        pt_pool = ctx.enter_context(tc.tile_pool(name="pt", bufs=1))
        on_pool = ctx.enter_context(tc.tile_pool(name="on", bufs=3))
        ot_pool = ctx.enter_context(tc.tile_pool(name="ot", bufs=4))
        rc_pool = ctx.enter_context(tc.tile_pool(name="rc", bufs=2))
        ysb_pool = ctx.enter_context(tc.tile_pool(name="ysb", bufs=3))
        ps_pool = ctx.enter_context(tc.tile_pool(name="ps", bufs=3, space="PSUM"))
        po_pool = ctx.enter_context(tc.tile_pool(name="po", bufs=2, space="PSUM"))

        def ps_tile(name):
            return ps_pool.tile([P, 2 * TCH], F32, name=name, tag="ps")

        # ---------------- load inputs ----------------
        # weights + first x column-chunk first, so phase 1/2 start early
        wqk_a = singles.tile([P, 2 * H * HS], BF16)
        nc.sync.dma_start(wqk_a, wqk[0:P, :])
        wqk_b = singles.tile([C - P, 2 * H * HS], BF16)
        nc.sync.dma_start(wqk_b, wqk[P:C, :])
        xT_a = singles.tile([P, T], BF16)
        nc.sync.dma_start(xT_a[:, 0:TCH], xT[0:P, 0:TCH])
        xT_b = singles.tile([C - P, T], BF16)
        nc.sync.dma_start(xT_b[:, 0:TCH], xT[P:C, 0:TCH])
        wv_a = singles.tile([P, H * HS], BF16)
        nc.sync.dma_start(wv_a, wv[0:P, :])
        wv_b = singles.tile([C - P, H * HS], BF16)
        nc.sync.dma_start(wv_b, wv[P:C, :])
        nc.sync.dma_start(xT_a[:, TCH:T], xT[0:P, TCH:T])
        nc.sync.dma_start(xT_b[:, TCH:T], xT[P:C, TCH:T])
        wp_a = singles.tile([P, C], BF16)
        nc.sync.dma_start(wp_a, wp[0:P, :])
        # second k-slab of Wproj lives at partition base 64 so it matches the
        # base of the overlapping DMA-transpose output OTy (d rows 64..191);
        # a base-0 copy (wp_b) serves the last chunk's PE-transpose path.
        wp_bb = singles.tile([P, C], BF16)
        nc.sync.dma_start(wp_bb[64:P, :], wp[P:H * HS, :])
        wp_b = singles.tile([H * HS - P, C], BF16)
        nc.sync.dma_start(wp_b, wp[P:H * HS, :])
        bp_sb = singles.tile([1, C], BF16)
        nc.sync.dma_start(bp_sb, bp[:, :])
        ident_sb = singles.tile([P, P], BF16)
        nc.sync.dma_start(ident_sb, ident[:, :])
        ones1 = singles.tile([1, P], BF16)
        nc.vector.memset(ones1, 1.0)

        # ---------------- phase 1 producers (emitted lazily) ----------------
        # PE requires fmap/weight at the same partition base, so q_h and k_h
        # live in separate tiles at identical row offsets.
        # wqk col order: [q0-3 (128) | k0-3 (128) | q4,q5 (64) | k4,k5 (64)]
        qkt = [singles.tile([P, T], BF16, name="qA"),
               singles.tile([P, T], BF16, name="kA"),
               singles.tile([P - 64, T], BF16, name="qB"),
               singles.tile([P - 64, T], BF16, name="kB")]
        qk_cols = [(0, P), (P, P), (2 * P, 64), (2 * P + 64, 64)]
        cp_engines = [nc.scalar.copy, nc.vector.tensor_copy]
        ci = [0]

        def emit_qk(g, c):
            c0, csz = qk_cols[g]
            ps = ps_tile("psqk")
            nc.tensor.matmul(ps[0:csz, 0:TCH], wqk_a[:, c0:c0 + csz],
                             xT_a[:, c * TCH:(c + 1) * TCH],
                             start=True, stop=False)
            nc.tensor.matmul(ps[0:csz, 0:TCH], wqk_b[:, c0:c0 + csz],
                             xT_b[:, c * TCH:(c + 1) * TCH],
                             start=False, stop=True)
            cp_engines[ci[0] % 2](qkt[g][:, c * TCH:(c + 1) * TCH],
                                  ps[0:csz, 0:TCH])
            ci[0] += 1

        va = [va_pool.tile([P, H * 33], BF16, name=f"va{si}")
              for si in range(NS)]

        def emit_va(si):
            ps = ps_tile("psv")
            nc.tensor.matmul(ps[:, 0:H * HS], xT_a[:, si * P:(si + 1) * P],
                             wv_a, start=True, stop=False)
            nc.tensor.matmul(ps[:, 0:H * HS], xT_b[:, si * P:(si + 1) * P],
                             wv_b, start=False, stop=True)
            va_r = va[si].rearrange("p (h e) -> p h e", h=H)
            ps_r = ps[:, 0:H * HS].rearrange("p (h d) -> p h d", h=H)
            cp_engines[ci[0] % 2](va_r[:, :, 0:HS], ps_r)
            ci[0] += 1
            nc.gpsimd.memset(va_r[:, :, 32], 1.0)

        # head -> (tile, row) maps; q_h and k_h share the same row base
        def qsrc(h):
            return (qkt[0], HS * h) if h < 4 else (qkt[2], HS * (h - 4))

        def ksrc(h):
            return (qkt[1], HS * h) if h < 4 else (qkt[3], HS * (h - 4))

        # chunk-0 interleave schedule: emit each producer just before its
        # first phase-2 consumer so the exp pipeline starts almost at t=0.
        sched = {}

        def sched_add(pr, fn, *args):
            sched.setdefault(pr, []).append((fn, args))

        sched_add(1, emit_qk, 1, 1)   # kA cols for si 4..7
        sched_add(5, emit_qk, 1, 2)   # si 8..11
        sched_add(9, emit_qk, 1, 3)   # si 12..15
        for si in range(1, NS):
            sched_add(si - 1, emit_va, si)
        sched_add(16, emit_qk, 2, 0)  # qB chunk 0 (heads 4,5 at pair 32)
        sched_add(18, emit_qk, 3, 0)  # kB cols for si 0..3
        sched_add(20, emit_qk, 3, 1)
        sched_add(22, emit_qk, 3, 2)
        sched_add(24, emit_qk, 3, 3)
        for i, (g, c) in enumerate(
                [(0, 1), (0, 2), (0, 3), (2, 1), (2, 2), (2, 3)]):
            sched_add(26 + 2 * i, emit_qk, g, c)  # later chunks' q columns

        # ---------------- phase 2 ----------------
        # ptp[p3][si]: P^T for head pair (2*p3, 2*p3+1); halves 512 cols each
        ptp = [[pt_pool.tile([P, 2 * TCH], BF16, name=f"pt{p3}_{si}")
                for si in range(NS)] for p3 in range(3)]

        # pre-seed: first QKT needs qA/kA chunk-0 columns and va[0]
        emit_qk(0, 0)
        emit_qk(1, 0)
        emit_va(0)

        def make_stagec(c, po2, last):
            """Stage C for chunk c as 3 task groups (emitted early in the
            NEXT chunk so the transpose latency hides under its QKTs)."""
            ons = []

            def norm_t(j):
                base = (j % 2) * 256
                po_r = po2[j // 2][:, base:base + H * 33].rearrange(
                    "p (h e) -> p h e", h=H)
                rcp = rc_pool.tile([P, 8], F32, name="rcp", tag="rc")
                nc.vector.reciprocal(rcp[:, 0:H], po_r[:, :, 32])
                on = on_pool.tile([P, C], BF16, name="on", tag="on")
                on_r = on[:, :].rearrange("p (h e) -> p h e", h=H)
                nc.vector.tensor_tensor(
                    on_r, po_r[:, :, 0:HS],
                    rcp[:, 0:H].to_broadcast([P, H, HS]), MUL)
                if not last:
                    # O^T via XBAR DMA transpose (SBUF->SBUF): d rows 0..127
                    # and (overlapping) 64..191, so both proj matmuls have
                    # matching fmap/weight partition bases (0 and 64).
                    otx = ot_pool.tile([P, P], BF16, name="otx", tag="otx")
                    oty = ot_pool.tile([P, P], BF16, name="oty", tag="oty")
                    nc.sync.dma_start_transpose(otx, on[:, 0:P])
                    nc.sync.dma_start_transpose(oty, on[:, 64:64 + P])
                else:
                    # tail chunk: PE transpose + engine copy (low latency)
                    psta = ps_tile("psta")
                    pstb = ps_tile("pstb")
                    nc.tensor.transpose(psta[:, 0:64].bitcast(BF16),
                                        on[:, 0:P], ident_sb)
                    nc.tensor.transpose(pstb[0:C - P, 0:64].bitcast(BF16),
                                        on[:, P:C], ident_sb)
                    otx = ot_pool.tile([P, P], BF16, name="otx", tag="otx")
                    oty = ot_pool.tile([C - P, P], BF16, name="otb",
                                       tag="otb")
                    nc.vector.tensor_copy(otx, psta[:, 0:64].bitcast(BF16))
                    nc.scalar.copy(oty, pstb[0:C - P, 0:64].bitcast(BF16))
                ons.append((otx, oty))

            def proj_t():
                psy2 = [po_pool.tile([P, TCH], F32, name=f"psy{jj}",
                                     tag=f"po{jj}", bufs=1)
                        for jj in range(2)]
                for jj in range(2):
                    for j in (2 * jj, 2 * jj + 1):
                        otx, oty = ons[j]
                        base = (j % 2) * 256
                        psy = psy2[jj][:, base:base + C]
                        nc.tensor.matmul(psy, otx, wp_a,
                                         start=(j % 2 == 0), stop=False)
                        if not last:
                            nc.tensor.matmul(
                                psy, oty[64:P, :], wp_bb[64:P, :],
                                start=False, stop=False,
                                tile_position=(64, 0),
                                skip_group_check=True)
                        else:
                            nc.tensor.matmul(psy, oty, wp_b,
                                             start=False, stop=False,
                                             tile_position=(0, 0),
                                             skip_group_check=True)
                        nc.tensor.matmul(psy, ones1, bp_sb,
                                         start=False, stop=(j % 2 == 1))
                    for j in (2 * jj, 2 * jj + 1):
                        t0 = c * TCH + j * P
                        base = (j % 2) * 256
                        ysb = ysb_pool.tile([P, C], F32, name="ysb", tag="y")
                        (nc.scalar.copy if j % 2 == 0
                         else nc.vector.tensor_copy)(
                            ysb, psy2[jj][:, base:base + C])
                        nc.sync.dma_start(out[t0:t0 + P, :], ysb)

            return [lambda: (norm_t(0), norm_t(1)),
                    lambda: (norm_t(2), norm_t(3)),
                    proj_t]

        pending_stagec = []
        for c in range(NCH):
            pairs = [(si, p3) for p3 in range(3) for si in range(NS)]
            npair = len(pairs)
            po2box = []

            def issue_pv(pr):
                po2 = po2box[0]
                si, p3 = pairs[pr]
                for half in (0, 1):
                    h = 2 * p3 + half
                    for j in range(NJ):
                        base = (j % 2) * 256
                        nc.tensor.matmul(
                            po2[j // 2][:, base + h * 33:base + (h + 1) * 33],
                            ptp[p3][si][:, half * TCH + j * P:
                                        half * TCH + (j + 1) * P],
                            va[si][:, h * 33:(h + 1) * 33],
                            start=(pr == 0 and half == 0 and j % 2 == 0),
                            stop=(pr == npair - 1 and half == 1
                                  and j % 2 == 1),
                            tile_position=(0, 0), skip_group_check=True)

            for pr, (si, p3) in enumerate(pairs):
                if c == 0:
                    for fn, args in sched.get(pr, []):
                        fn(*args)
                if pending_stagec and pr < len(pending_stagec):
                    pending_stagec[pr]()
                if pr == LAG - 1:
                    # the previous chunk's stage C is fully traced; now the
                    # PSUM bank tags can rotate to this chunk's accumulators
                    pending_stagec = []
                    po2box.append(
                        [po_pool.tile([P, TCH], F32, name=f"po{jj}",
                                      tag=f"po{jj}", bufs=1)
                         for jj in range(2)])
                ps = ps_tile("psst")
                for half in (0, 1):
                    h = 2 * p3 + half
                    kt, kr = ksrc(h)
                    qt, qr = qsrc(h)
                    nc.tensor.matmul(
                        ps[:, half * TCH:(half + 1) * TCH],
                        kt[kr:kr + HS, si * P:(si + 1) * P],
                        qt[qr:qr + HS, c * TCH:(c + 1) * TCH],
                        start=True, stop=True,
                        tile_position=(kr, 0), skip_group_check=True)
                eng = EXP_PAT[pr]
                dst = ptp[p3][si]
                if eng == 0:
                    nc.scalar.activation(dst, ps, Exp, scale=SCALE)
                else:
                    nc.vector.tensor_scalar(dst[:, :].bitcast(I16), ps,
                                            AEXP, BEXP, MUL, ADD)
                if pr >= LAG:
                    issue_pv(pr - LAG)
            for pr in range(npair - LAG, npair):
                issue_pv(pr)

            last = (c == NCH - 1)
            tasks = make_stagec(c, po2box[0], last)
            if last:
                for t in tasks:
                    t()
            else:
                pending_stagec = tasks

    nc.compile()
    return nc


def _get_nc():
    if "nc" not in _CACHE:
        _CACHE["nc"] = build_nc()
    return _CACHE["nc"]


def make_in_maps(x, Wq, Wk, Wv, Wproj, bproj):
    bf = ml_dtypes.bfloat16
    x = np.asarray(x, np.float32)
    pack = lambda w: np.ascontiguousarray(
        np.transpose(np.asarray(w, np.float32), (1, 0, 2)).reshape(C, H * HS))
    pq, pk = pack(Wq), pack(Wk)
    # col order: q0-3 | k0-3 | q4,q5 | k4,k5 (q_h/k_h at equal row bases)
    wqk = np.ascontiguousarray(np.concatenate(
        [pq[:, 0:128], pk[:, 0:128], pq[:, 128:192], pk[:, 128:192]],
        axis=1)).astype(bf)
    wv = pack(Wv).astype(bf)
    wp = np.ascontiguousarray(np.asarray(Wproj, np.float32)).astype(bf)
    bp = np.asarray(bproj, np.float32).reshape(1, C).astype(bf)
    ident = np.eye(P, dtype=np.float32).astype(bf)
    maps = []
    for i in range(B):
        xti = np.ascontiguousarray(x[i].T).astype(bf)
        maps.append({"xT": xti, "wqk": wqk, "wv": wv, "wp": wp, "bp": bp,
                     "ident": ident})
    return maps


def run(inputs, trace=False, **kw):
    nc = _get_nc()
    in_maps = make_in_maps(**inputs)
    res = run_bass_kernel_spmd(nc, in_maps, core_ids=list(range(B)),
                               trace=trace, **kw)
    y = np.stack([np.asarray(res.results[i]["out"], np.float32)
                  for i in range(B)], axis=0)
    return y, res


def kernel(**inputs):
    y, _ = run(inputs, trace=False)
    return y


# revision 46
# speedup vs baseline: 1.0074x; 1.0014x over previous
"""Multi-head self-attention (B=8, T=2048, C=192, H=6, HS=32) on 8 TRN2 cores.

Sharding: data-parallel over batch - core i computes batch element i fully
on-chip (no collectives). Host pre-transposes x and packs weights.

Per core:
  qkT [384, t]  = wqk.T @ x.T, packed into 3x[128, T] tiles (q0-3 | q4,q5,k0,k1 | k2-5)
  v_aug [s, 6*33] = x @ Wv, per-head [v_h | ones] (33 cols)
  S^T [s, t]    = kT_h.T @ qT_h        (per (si, h), one 512-wide matmul)
  P^T           = exp(S^T / sqrt(HS))  split across 3 engines:
                    ScalarE: exact exp activation
                    DVE/GPSIMD: Schraudolph exp via int16 bit trick
                      bf16bits(exp(s)) ~ int16(s*AEXP + BEXP)
  O[t, d+sum]   = PV with P^T as STATIONARY (cost ~ 33/row vs 512/row)
  normalize per-partition 1/rowsum (tensor_scalar with per-partition scalar)
  O^T via PE transpose; Y[t, c] = O^T.T @ Wproj + bias; DMA out.
"""

import numpy as np
import ml_dtypes
from collections import deque
from contextlib import ExitStack

import concourse.bass as bass
import concourse.tile as tile
from concourse import bacc, mybir
from concourse.bass_utils import run_bass_kernel_spmd

B, T, C = 8, 2048, 192
H, HS = 6, 32
P = 128
TCH = 512            # t-chunk width (one PSUM bank of fp32)
NCH = T // TCH       # 4
NS = T // P          # 16 s-tiles
NJ = TCH // P        # 4 t-tiles per chunk
SCALE = 1.0 / float(np.sqrt(HS))
BF16 = mybir.dt.bfloat16
F32 = mybir.dt.float32
I16 = mybir.dt.int16
Exp = mybir.ActivationFunctionType.Exp
MUL = mybir.AluOpType.mult
ADD = mybir.AluOpType.add

# Schraudolph exp in bf16-bit domain: bf16_bits = int16(s * AEXP + BEXP)
AEXP = float(SCALE * 128.0 * np.log2(np.e))
BEXP = 16248.5

# per-chunk engine split for the 48 exp pair-tiles (GPSIMD cannot access
# PSUM, so only ScalarE (exact exp) and DVE (Schraudolph) share the work).
# The first 10 pairs are scalar-heavy: DVE digests the previous chunk's
# stage-C burst (recip/normalize/output copies) there.
def _mk_pattern():
    head = [0, 0, 0, 1, 0, 0, 0, 1, 0, 0]  # 8 scalar : 2 dve
    rest = sorted([((i + 0.5) / 19, 0) for i in range(19)]
                  + [((i + 0.5) / 19, 1) for i in range(19)])
    return head + [e for _, e in rest]      # totals: 27 scalar, 21 dve


EXP_PAT = _mk_pattern()
LAG = 3  # software pipeline depth (in pair-steps) between QKT and PV

_CACHE = {}


def build_nc():
    nc = bacc.Bacc()
    xT = nc.declare_dram_parameter("xT", [C, T], BF16, isOutput=False)
    wqk = nc.declare_dram_parameter("wqk", [C, 2 * H * HS], BF16, isOutput=False)
    wv = nc.declare_dram_parameter("wv", [C, H * HS], BF16, isOutput=False)
    wp = nc.declare_dram_parameter("wp", [H * HS, C], BF16, isOutput=False)
    bp = nc.declare_dram_parameter("bp", [1, C], BF16, isOutput=False)
    ident = nc.declare_dram_parameter("ident", [P, P], BF16, isOutput=False)
    out = nc.declare_dram_parameter("out", [T, C], F32, isOutput=True)

    with tile.TileContext(nc) as tc, ExitStack() as ctx:
        singles = ctx.enter_context(tc.tile_pool(name="singles", bufs=1))
        qk_pool = ctx.enter_context(tc.tile_pool(name="qk", bufs=1))
        va_pool = ctx.enter_context(tc.tile_pool(name="va", bufs=1))
        pt_pool = ctx.enter_context(tc.tile_pool(name="pt", bufs=1))
        on_pool = ctx.enter_context(tc.tile_pool(name="on", bufs=3))
        ot_pool = ctx.enter_context(tc.tile_pool(name="ot", bufs=4))
        rc_pool = ctx.enter_context(tc.tile_pool(name="rc", bufs=2))
        ysb_pool = ctx.enter_context(tc.tile_pool(name="ysb", bufs=3))
        ps_pool = ctx.enter_context(tc.tile_pool(name="ps", bufs=3, space="PSUM"))
        po_pool = ctx.enter_context(tc.tile_pool(name="po", bufs=2, space="PSUM"))

        def ps_tile(name):
            return ps_pool.tile([P, 2 * TCH], F32, name=name, tag="ps")

        # ---------------- load inputs ----------------
        # weights + first x column-chunk first, so phase 1/2 start early
        wqk_a = singles.tile([P, 2 * H * HS], BF16)
        nc.sync.dma_start(wqk_a, wqk[0:P, :])
        wqk_b = singles.tile([C - P, 2 * H * HS], BF16)
        nc.sync.dma_start(wqk_b, wqk[P:C, :])
        xT_a = singles.tile([P, T], BF16)
        nc.sync.dma_start(xT_a[:, 0:TCH], xT[0:P, 0:TCH])
        xT_b = singles.tile([C - P, T], BF16)
        nc.sync.dma_start(xT_b[:, 0:TCH], xT[P:C, 0:TCH])
        wv_a = singles.tile([P, H * HS], BF16)
        nc.sync.dma_start(wv_a, wv[0:P, :])
        wv_b = singles.tile([C - P, H * HS], BF16)
        nc.sync.dma_start(wv_b, wv[P:C, :])
        nc.sync.dma_start(xT_a[:, TCH:T], xT[0:P, TCH:T])
        nc.sync.dma_start(xT_b[:, TCH:T], xT[P:C, TCH:T])
        wp_a = singles.tile([P, C], BF16)
        nc.sync.dma_start(wp_a, wp[0:P, :])
        # second k-slab of Wproj lives at partition base 64 so it matches the
        # base of the overlapping DMA-transpose output OTy (d rows 64..191);
        # a base-0 copy (wp_b) serves the last chunk's PE-transpose path.
        wp_bb = singles.tile([P, C], BF16)
        nc.sync.dma_start(wp_bb[64:P, :], wp[P:H * HS, :])
        wp_b = singles.tile([H * HS - P, C], BF16)
        nc.sync.dma_start(wp_b, wp[P:H * HS, :])
        bp_sb = singles.tile([1, C], BF16)
        nc.sync.dma_start(bp_sb, bp[:, :])
        ident_sb = singles.tile([P, P], BF16)
        nc.sync.dma_start(ident_sb, ident[:, :])
        ones1 = singles.tile([1, P], BF16)
        nc.vector.memset(ones1, 1.0)

        # ---------------- phase 1 producers (emitted lazily) ----------------
        # PE requires fmap/weight at the same partition base, so q_h and k_h
        # live in separate tiles at identical row offsets.
        # wqk col order: [q0-3 (128) | k0-3 (128) | q4,q5 (64) | k4,k5 (64)]
        qkt = [singles.tile([P, T], BF16, name="qA"),
               singles.tile([P, T], BF16, name="kA"),
               singles.tile([P - 64, T], BF16, name="qB"),
               singles.tile([P - 64, T], BF16, name="kB")]
        qk_cols = [(0, P), (P, P), (2 * P, 64), (2 * P + 64, 64)]
        cp_engines = [nc.scalar.copy, nc.vector.tensor_copy]
        ci = [0]

        def emit_qk(g, c):
            c0, csz = qk_cols[g]
            ps = ps_tile("psqk")
            nc.tensor.matmul(ps[0:csz, 0:TCH], wqk_a[:, c0:c0 + csz],
                             xT_a[:, c * TCH:(c + 1) * TCH],
                             start=True, stop=False)
            nc.tensor.matmul(ps[0:csz, 0:TCH], wqk_b[:, c0:c0 + csz],
                             xT_b[:, c * TCH:(c + 1) * TCH],
                             start=False, stop=True)
            cp_engines[ci[0] % 2](qkt[g][:, c * TCH:(c + 1) * TCH],
                                  ps[0:csz, 0:TCH])
            ci[0] += 1

        va = [va_pool.tile([P, H * 33], BF16, name=f"va{si}")
              for si in range(NS)]

        def emit_va(si):
            ps = ps_tile("psv")
            nc.tensor.matmul(ps[:, 0:H * HS], xT_a[:, si * P:(si + 1) * P],
                             wv_a, start=True, stop=False)
            nc.tensor.matmul(ps[:, 0:H * HS], xT_b[:, si * P:(si + 1) * P],
                             wv_b, start=False, stop=True)
            va_r = va[si].rearrange("p (h e) -> p h e", h=H)
            ps_r = ps[:, 0:H * HS].rearrange("p (h d) -> p h d", h=H)
            cp_engines[ci[0] % 2](va_r[:, :, 0:HS], ps_r)
            ci[0] += 1
            nc.gpsimd.memset(va_r[:, :, 32], 1.0)

        # head -> (tile, row) maps; q_h and k_h share the same row base
        def qsrc(h):
            return (qkt[0], HS * h) if h < 4 else (qkt[2], HS * (h - 4))

        def ksrc(h):
            return (qkt[1], HS * h) if h < 4 else (qkt[3], HS * (h - 4))

        # chunk-0 interleave schedule: emit each producer just before its
        # first phase-2 consumer so the exp pipeline starts almost at t=0.
        sched = {}

        def sched_add(pr, fn, *args):
            sched.setdefault(pr, []).append((fn, args))

        sched_add(1, emit_qk, 1, 1)   # kA cols for si 4..7
        sched_add(5, emit_qk, 1, 2)   # si 8..11
        sched_add(9, emit_qk, 1, 3)   # si 12..15
        for si in range(1, NS):
            sched_add(si - 1, emit_va, si)
        sched_add(16, emit_qk, 2, 0)  # qB chunk 0 (heads 4,5 at pair 32)
        sched_add(18, emit_qk, 3, 0)  # kB cols for si 0..3
        sched_add(20, emit_qk, 3, 1)
        sched_add(22, emit_qk, 3, 2)
        sched_add(24, emit_qk, 3, 3)
        for i, (g, c) in enumerate(
                [(0, 1), (0, 2), (0, 3), (2, 1), (2, 2), (2, 3)]):
            sched_add(26 + 2 * i, emit_qk, g, c)  # later chunks' q columns

        # ---------------- phase 2 ----------------
        # ptp[p3][si]: P^T for head pair (2*p3, 2*p3+1); halves 512 cols each
        ptp = [[pt_pool.tile([P, 2 * TCH], BF16, name=f"pt{p3}_{si}")
                for si in range(NS)] for p3 in range(3)]

        # pre-seed: first QKT needs qA/kA chunk-0 columns and va[0]
        emit_qk(0, 0)
        emit_qk(1, 0)
        emit_va(0)

        def make_stagec(c, po2, last):
            """Stage C for chunk c as 3 task groups (emitted early in the
            NEXT chunk so the transpose latency hides under its QKTs)."""
            ons = []

            def norm_t(j):
                base = (j % 2) * 256
                po_r = po2[j // 2][:, base:base + H * 33].rearrange(
                    "p (h e) -> p h e", h=H)
                rcp = rc_pool.tile([P, 8], F32, name="rcp", tag="rc")
                nc.vector.reciprocal(rcp[:, 0:H], po_r[:, :, 32])
                on = on_pool.tile([P, C], BF16, name="on", tag="on")
                on_r = on[:, :].rearrange("p (h e) -> p h e", h=H)
                nc.vector.tensor_tensor(
                    on_r, po_r[:, :, 0:HS],
                    rcp[:, 0:H].to_broadcast([P, H, HS]), MUL)
                if not last:
                    # O^T via XBAR DMA transpose (SBUF->SBUF): d rows 0..127
                    # and (overlapping) 64..191, so both proj matmuls have
                    # matching fmap/weight partition bases (0 and 64).
                    otx = ot_pool.tile([P, P], BF16, name="otx", tag="otx")
                    oty = ot_pool.tile([P, P], BF16, name="oty", tag="oty")
                    nc.sync.dma_start_transpose(otx, on[:, 0:P])
                    nc.sync.dma_start_transpose(oty, on[:, 64:64 + P])
                else:
                    # tail chunk: PE transpose + engine copy (low latency)
                    psta = ps_tile("psta")
                    pstb = ps_tile("pstb")
                    nc.tensor.transpose(psta[:, 0:64].bitcast(BF16),
                                        on[:, 0:P], ident_sb)
                    nc.tensor.transpose(pstb[0:C - P, 0:64].bitcast(BF16),
                                        on[:, P:C], ident_sb)
                    otx = ot_pool.tile([P, P], BF16, name="otx", tag="otx")
                    oty = ot_pool.tile([C - P, P], BF16, name="otb",
                                       tag="otb")
                    nc.vector.tensor_copy(otx, psta[:, 0:64].bitcast(BF16))
                    nc.scalar.copy(oty, pstb[0:C - P, 0:64].bitcast(BF16))
                ons.append((otx, oty))

            def proj_t():
                psy2 = [po_pool.tile([P, TCH], F32, name=f"psy{jj}",
                                     tag=f"po{jj}", bufs=1)
                        for jj in range(2)]
                for jj in range(2):
                    for j in (2 * jj, 2 * jj + 1):
                        otx, oty = ons[j]
                        base = (j % 2) * 256
                        psy = psy2[jj][:, base:base + C]
                        nc.tensor.matmul(psy, otx, wp_a,
                                         start=(j % 2 == 0), stop=False)
                        if not last:
                            nc.tensor.matmul(
                                psy, oty[64:P, :], wp_bb[64:P, :],
                                start=False, stop=False,
                                tile_position=(64, 0),
                                skip_group_check=True)
                        else:
                            nc.tensor.matmul(psy, oty, wp_b,
                                             start=False, stop=False,
                                             tile_position=(0, 0),
                                             skip_group_check=True)
                        nc.tensor.matmul(psy, ones1, bp_sb,
                                         start=False, stop=(j % 2 == 1))
                    for j in (2 * jj, 2 * jj + 1):
                        t0 = c * TCH + j * P
                        base = (j % 2) * 256
                        ysb = ysb_pool.tile([P, C], F32, name="ysb", tag="y")
                        (nc.scalar.copy if j % 2 == 0
                         else nc.vector.tensor_copy)(
                            ysb, psy2[jj][:, base:base + C])
                        nc.sync.dma_start(out[t0:t0 + P, :], ysb)

            return [lambda: (norm_t(0), norm_t(1)),
                    lambda: (norm_t(2), norm_t(3)),
                    proj_t]

        pairs = [(si, p3) for p3 in range(3) for si in range(NS)]
        npair = len(pairs)

        def emit_qkt_exp(c, pr):
            si, p3 = pairs[pr]
            ps = ps_tile("psst")
            for half in (0, 1):
                h = 2 * p3 + half
                kt, kr = ksrc(h)
                qt, qr = qsrc(h)
                nc.tensor.matmul(
                    ps[:, half * TCH:(half + 1) * TCH],
                    kt[kr:kr + HS, si * P:(si + 1) * P],
                    qt[qr:qr + HS, c * TCH:(c + 1) * TCH],
                    start=True, stop=True,
                    tile_position=(kr, 0), skip_group_check=True)
            dst = ptp[p3][si]
            if EXP_PAT[pr] == 0:
                nc.scalar.activation(dst, ps, Exp, scale=SCALE)
            else:
                nc.vector.tensor_scalar(dst[:, :].bitcast(I16), ps,
                                        AEXP, BEXP, MUL, ADD)

        pending_stagec = []
        pre_emitted = False
        for c in range(NCH):
            po2box = []

            def issue_pv(pr):
                po2 = po2box[0]
                si, p3 = pairs[pr]
                for half in (0, 1):
                    h = 2 * p3 + half
                    for j in range(NJ):
                        base = (j % 2) * 256
                        nc.tensor.matmul(
                            po2[j // 2][:, base + h * 33:base + (h + 1) * 33],
                            ptp[p3][si][:, half * TCH + j * P:
                                        half * TCH + (j + 1) * P],
                            va[si][:, h * 33:(h + 1) * 33],
                            start=(pr == 0 and half == 0 and j % 2 == 0),
                            stop=(pr == npair - 1 and half == 1
                                  and j % 2 == 1),
                            tile_position=(0, 0), skip_group_check=True)

            for pr, (si, p3) in enumerate(pairs):
                if c == 0:
                    for fn, args in sched.get(pr, []):
                        fn(*args)
                if pending_stagec and pr < len(pending_stagec):
                    pending_stagec[pr]()
                if pr == LAG - 1:
                    # the previous chunk's stage C is fully traced; now the
                    # PSUM bank tags can rotate to this chunk's accumulators
                    pending_stagec = []
                    po2box.append(
                        [po_pool.tile([P, TCH], F32, name=f"po{jj}",
                                      tag=f"po{jj}", bufs=1)
                         for jj in range(2)])
                if not (pre_emitted and pr < LAG):
                    emit_qkt_exp(c, pr)
                if pr >= LAG:
                    issue_pv(pr - LAG)
            # prime the next chunk's exp pipeline before this chunk's PV
            # tail so the boundary never drains the engines
            if c + 1 < NCH:
                for pr2 in range(LAG):
                    emit_qkt_exp(c + 1, pr2)
                pre_emitted = True
            for pr in range(npair - LAG, npair):
                issue_pv(pr)

            last = (c == NCH - 1)
            tasks = make_stagec(c, po2box[0], last)
            if last:
                for t in tasks:
                    t()
            else:
                pending_stagec = tasks

    nc.compile()
    return nc


def _get_nc():
    if "nc" not in _CACHE:
        _CACHE["nc"] = build_nc()
    return _CACHE["nc"]


def make_in_maps(x, Wq, Wk, Wv, Wproj, bproj):
    bf = ml_dtypes.bfloat16
    x = np.asarray(x, np.float32)
    pack = lambda w: np.ascontiguousarray(
        np.transpose(np.asarray(w, np.float32), (1, 0, 2)).reshape(C, H * HS))
    pq, pk = pack(Wq), pack(Wk)
    # col order: q0-3 | k0-3 | q4,q5 | k4,k5 (q_h/k_h at equal row bases)
    wqk = np.ascontiguousarray(np.concatenate(
        [pq[:, 0:128], pk[:, 0:128], pq[:, 128:192], pk[:, 128:192]],
        axis=1)).astype(bf)
    wv = pack(Wv).astype(bf)
    wp = np.ascontiguousarray(np.asarray(Wproj, np.float32)).astype(bf)
    bp = np.asarray(bproj, np.float32).reshape(1, C).astype(bf)
    ident = np.eye(P, dtype=np.float32).astype(bf)
    maps = []
    for i in range(B):
        xti = np.ascontiguousarray(x[i].T).astype(bf)
        maps.append({"xT": xti, "wqk": wqk, "wv": wv, "wp": wp, "bp": bp,
                     "ident": ident})
    return maps


def run(inputs, trace=False, **kw):
    nc = _get_nc()
    in_maps = make_in_maps(**inputs)
    res = run_bass_kernel_spmd(nc, in_maps, core_ids=list(range(B)),
                               trace=trace, **kw)
    y = np.stack([np.asarray(res.results[i]["out"], np.float32)
                  for i in range(B)], axis=0)
    return y, res


def kernel(**inputs):
    y, _ = run(inputs, trace=False)
    return y


# revision 48
# speedup vs baseline: 1.0107x; 1.0033x over previous
"""Multi-head self-attention (B=8, T=2048, C=192, H=6, HS=32) on 8 TRN2 cores.

Sharding: data-parallel over batch - core i computes batch element i fully
on-chip (no collectives). Host pre-transposes x and packs weights.

Per core:
  qkT [384, t]  = wqk.T @ x.T, packed into 3x[128, T] tiles (q0-3 | q4,q5,k0,k1 | k2-5)
  v_aug [s, 6*33] = x @ Wv, per-head [v_h | ones] (33 cols)
  S^T [s, t]    = kT_h.T @ qT_h        (per (si, h), one 512-wide matmul)
  P^T           = exp(S^T / sqrt(HS))  split across 3 engines:
                    ScalarE: exact exp activation
                    DVE/GPSIMD: Schraudolph exp via int16 bit trick
                      bf16bits(exp(s)) ~ int16(s*AEXP + BEXP)
  O[t, d+sum]   = PV with P^T as STATIONARY (cost ~ 33/row vs 512/row)
  normalize per-partition 1/rowsum (tensor_scalar with per-partition scalar)
  O^T via PE transpose; Y[t, c] = O^T.T @ Wproj + bias; DMA out.
"""

import numpy as np
import ml_dtypes
from collections import deque
from contextlib import ExitStack

import concourse.bass as bass
import concourse.tile as tile
from concourse import bacc, mybir
from concourse.bass_utils import run_bass_kernel_spmd

B, T, C = 8, 2048, 192
H, HS = 6, 32
P = 128
TCH = 512            # t-chunk width (one PSUM bank of fp32)
NCH = T // TCH       # 4
NS = T // P          # 16 s-tiles
NJ = TCH // P        # 4 t-tiles per chunk
SCALE = 1.0 / float(np.sqrt(HS))
BF16 = mybir.dt.bfloat16
F32 = mybir.dt.float32
I16 = mybir.dt.int16
Exp = mybir.ActivationFunctionType.Exp
MUL = mybir.AluOpType.mult
ADD = mybir.AluOpType.add

# Schraudolph exp in bf16-bit domain: bf16_bits = int16(s * AEXP + BEXP)
AEXP = float(SCALE * 128.0 * np.log2(np.e))
BEXP = 16248.5

# per-chunk engine split for the 48 exp pair-tiles (GPSIMD cannot access
# PSUM, so only ScalarE (exact exp) and DVE (Schraudolph) share the work).
# The first 10 pairs are scalar-heavy: DVE digests the previous chunk's
# stage-C burst (recip/normalize/output copies) there.
def _mk_pattern():
    head = [0, 0, 0, 1, 0, 0, 0, 1, 0, 0]  # 8 scalar : 2 dve
    rest = sorted([((i + 0.5) / 19, 0) for i in range(19)]
                  + [((i + 0.5) / 19, 1) for i in range(19)])
    return head + [e for _, e in rest]      # totals: 27 scalar, 21 dve


EXP_PAT = _mk_pattern()
LAG = 3  # software pipeline depth (in pair-steps) between QKT and PV

_CACHE = {}


def build_nc():
    nc = bacc.Bacc()
    xT = nc.declare_dram_parameter("xT", [C, T], BF16, isOutput=False)
    wqk = nc.declare_dram_parameter("wqk", [C, 2 * H * HS], BF16, isOutput=False)
    wv = nc.declare_dram_parameter("wv", [C, H * HS], BF16, isOutput=False)
    wp = nc.declare_dram_parameter("wp", [H * HS, C], BF16, isOutput=False)
    bp = nc.declare_dram_parameter("bp", [1, C], BF16, isOutput=False)
    ident = nc.declare_dram_parameter("ident", [P, P], BF16, isOutput=False)
    out = nc.declare_dram_parameter("out", [T, C], F32, isOutput=True)

    with tile.TileContext(nc) as tc, ExitStack() as ctx:
        singles = ctx.enter_context(tc.tile_pool(name="singles", bufs=1))
        qk_pool = ctx.enter_context(tc.tile_pool(name="qk", bufs=1))
        va_pool = ctx.enter_context(tc.tile_pool(name="va", bufs=1))
        pt_pool = ctx.enter_context(tc.tile_pool(name="pt", bufs=1))
        on_pool = ctx.enter_context(tc.tile_pool(name="on", bufs=3))
        ot_pool = ctx.enter_context(tc.tile_pool(name="ot", bufs=4))
        rc_pool = ctx.enter_context(tc.tile_pool(name="rc", bufs=2))
        ysb_pool = ctx.enter_context(tc.tile_pool(name="ysb", bufs=3))
        ps_pool = ctx.enter_context(tc.tile_pool(name="ps", bufs=3, space="PSUM"))
        po_pool = ctx.enter_context(tc.tile_pool(name="po", bufs=2, space="PSUM"))

        def ps_tile(name):
            return ps_pool.tile([P, 2 * TCH], F32, name=name, tag="ps")

        # ---------------- load inputs ----------------
        # weights + first x column-chunk first, issued from three different
        # sequencer queues in parallel, so phase 1/2 start early
        wqk_a = singles.tile([P, 2 * H * HS], BF16)
        nc.sync.dma_start(wqk_a, wqk[0:P, :])
        wqk_b = singles.tile([C - P, 2 * H * HS], BF16)
        nc.sync.dma_start(wqk_b, wqk[P:C, :])
        xT_a = singles.tile([P, T], BF16)
        nc.scalar.dma_start(xT_a[:, 0:TCH], xT[0:P, 0:TCH])
        xT_b = singles.tile([C - P, T], BF16)
        nc.gpsimd.dma_start(xT_b[:, 0:TCH], xT[P:C, 0:TCH])
        # warm the Exp activation table while DMAs are in flight, so the
        # first real exp doesn't pay the 1.3us table load
        warm = singles.tile([1, 8], F32)
        nc.vector.memset(warm, 1.0)
        nc.scalar.activation(warm[0:1, 4:6], warm[0:1, 0:2], Exp)
        wv_a = singles.tile([P, H * HS], BF16)
        nc.sync.dma_start(wv_a, wv[0:P, :])
        wv_b = singles.tile([C - P, H * HS], BF16)
        nc.sync.dma_start(wv_b, wv[P:C, :])
        nc.sync.dma_start(xT_a[:, TCH:T], xT[0:P, TCH:T])
        nc.sync.dma_start(xT_b[:, TCH:T], xT[P:C, TCH:T])
        wp_a = singles.tile([P, C], BF16)
        nc.sync.dma_start(wp_a, wp[0:P, :])
        # second k-slab of Wproj lives at partition base 64 so it matches the
        # base of the overlapping DMA-transpose output OTy (d rows 64..191);
        # a base-0 copy (wp_b) serves the last chunk's PE-transpose path.
        wp_bb = singles.tile([P, C], BF16)
        nc.sync.dma_start(wp_bb[64:P, :], wp[P:H * HS, :])
        wp_b = singles.tile([H * HS - P, C], BF16)
        nc.sync.dma_start(wp_b, wp[P:H * HS, :])
        bp_sb = singles.tile([1, C], BF16)
        nc.sync.dma_start(bp_sb, bp[:, :])
        ident_sb = singles.tile([P, P], BF16)
        nc.sync.dma_start(ident_sb, ident[:, :])
        ones1 = singles.tile([1, P], BF16)
        nc.vector.memset(ones1, 1.0)

        # ---------------- phase 1 producers (emitted lazily) ----------------
        # PE requires fmap/weight at the same partition base, so q_h and k_h
        # live in separate tiles at identical row offsets.
        # wqk col order: [q0-3 (128) | k0-3 (128) | q4,q5 (64) | k4,k5 (64)]
        qkt = [singles.tile([P, T], BF16, name="qA"),
               singles.tile([P, T], BF16, name="kA"),
               singles.tile([P - 64, T], BF16, name="qB"),
               singles.tile([P - 64, T], BF16, name="kB")]
        qk_cols = [(0, P), (P, P), (2 * P, 64), (2 * P + 64, 64)]
        cp_engines = [nc.scalar.copy, nc.vector.tensor_copy]
        ci = [0]

        def emit_qk(g, c):
            c0, csz = qk_cols[g]
            ps = ps_tile("psqk")
            nc.tensor.matmul(ps[0:csz, 0:TCH], wqk_a[:, c0:c0 + csz],
                             xT_a[:, c * TCH:(c + 1) * TCH],
                             start=True, stop=False)
            nc.tensor.matmul(ps[0:csz, 0:TCH], wqk_b[:, c0:c0 + csz],
                             xT_b[:, c * TCH:(c + 1) * TCH],
                             start=False, stop=True)
            cp_engines[ci[0] % 2](qkt[g][:, c * TCH:(c + 1) * TCH],
                                  ps[0:csz, 0:TCH])
            ci[0] += 1

        va = [va_pool.tile([P, H * 33], BF16, name=f"va{si}")
              for si in range(NS)]

        def emit_va(si):
            ps = ps_tile("psv")
            nc.tensor.matmul(ps[:, 0:H * HS], xT_a[:, si * P:(si + 1) * P],
                             wv_a, start=True, stop=False)
            nc.tensor.matmul(ps[:, 0:H * HS], xT_b[:, si * P:(si + 1) * P],
                             wv_b, start=False, stop=True)
            va_r = va[si].rearrange("p (h e) -> p h e", h=H)
            ps_r = ps[:, 0:H * HS].rearrange("p (h d) -> p h d", h=H)
            cp_engines[ci[0] % 2](va_r[:, :, 0:HS], ps_r)
            ci[0] += 1
            nc.gpsimd.memset(va_r[:, :, 32], 1.0)

        # head -> (tile, row) maps; q_h and k_h share the same row base
        def qsrc(h):
            return (qkt[0], HS * h) if h < 4 else (qkt[2], HS * (h - 4))

        def ksrc(h):
            return (qkt[1], HS * h) if h < 4 else (qkt[3], HS * (h - 4))

        # chunk-0 interleave schedule: emit each producer just before its
        # first phase-2 consumer so the exp pipeline starts almost at t=0.
        sched = {}

        def sched_add(pr, fn, *args):
            sched.setdefault(pr, []).append((fn, args))

        sched_add(1, emit_qk, 1, 1)   # kA cols for si 4..7
        sched_add(5, emit_qk, 1, 2)   # si 8..11
        sched_add(9, emit_qk, 1, 3)   # si 12..15
        for si in range(1, NS):
            sched_add(si - 1, emit_va, si)
        sched_add(16, emit_qk, 2, 0)  # qB chunk 0 (heads 4,5 at pair 32)
        sched_add(18, emit_qk, 3, 0)  # kB cols for si 0..3
        sched_add(20, emit_qk, 3, 1)
        sched_add(22, emit_qk, 3, 2)
        sched_add(24, emit_qk, 3, 3)
        for i, (g, c) in enumerate(
                [(0, 1), (0, 2), (0, 3), (2, 1), (2, 2), (2, 3)]):
            sched_add(26 + 2 * i, emit_qk, g, c)  # later chunks' q columns

        # ---------------- phase 2 ----------------
        # ptp[p3][si]: P^T for head pair (2*p3, 2*p3+1); halves 512 cols each
        ptp = [[pt_pool.tile([P, 2 * TCH], BF16, name=f"pt{p3}_{si}")
                for si in range(NS)] for p3 in range(3)]

        # pre-seed: first QKT needs qA/kA chunk-0 columns and va[0]
        emit_qk(0, 0)
        emit_qk(1, 0)
        emit_va(0)

        def make_stagec(c, po2, last):
            """Stage C for chunk c as 3 task groups (emitted early in the
            NEXT chunk so the transpose latency hides under its QKTs)."""
            ons = []

            def norm_t(j):
                base = (j % 2) * 256
                po_r = po2[j // 2][:, base:base + H * 33].rearrange(
                    "p (h e) -> p h e", h=H)
                rcp = rc_pool.tile([P, 8], F32, name="rcp", tag="rc")
                nc.vector.reciprocal(rcp[:, 0:H], po_r[:, :, 32])
                on = on_pool.tile([P, C], BF16, name="on", tag="on")
                on_r = on[:, :].rearrange("p (h e) -> p h e", h=H)
                nc.vector.tensor_tensor(
                    on_r, po_r[:, :, 0:HS],
                    rcp[:, 0:H].to_broadcast([P, H, HS]), MUL)
                if not last:
                    # O^T via XBAR DMA transpose (SBUF->SBUF): d rows 0..127
                    # and (overlapping) 64..191, so both proj matmuls have
                    # matching fmap/weight partition bases (0 and 64).
                    otx = ot_pool.tile([P, P], BF16, name="otx", tag="otx")
                    oty = ot_pool.tile([P, P], BF16, name="oty", tag="oty")
                    nc.sync.dma_start_transpose(otx, on[:, 0:P])
                    nc.sync.dma_start_transpose(oty, on[:, 64:64 + P])
                else:
                    # tail chunk: PE transpose + engine copy (low latency)
                    psta = ps_tile("psta")
                    pstb = ps_tile("pstb")
                    nc.tensor.transpose(psta[:, 0:64].bitcast(BF16),
                                        on[:, 0:P], ident_sb)
                    nc.tensor.transpose(pstb[0:C - P, 0:64].bitcast(BF16),
                                        on[:, P:C], ident_sb)
                    otx = ot_pool.tile([P, P], BF16, name="otx", tag="otx")
                    oty = ot_pool.tile([C - P, P], BF16, name="otb",
                                       tag="otb")
                    nc.vector.tensor_copy(otx, psta[:, 0:64].bitcast(BF16))
                    nc.scalar.copy(oty, pstb[0:C - P, 0:64].bitcast(BF16))
                ons.append((otx, oty))

            def proj_t():
                psy2 = [po_pool.tile([P, TCH], F32, name=f"psy{jj}",
                                     tag=f"po{jj}", bufs=1)
                        for jj in range(2)]
                for jj in range(2):
                    for j in (2 * jj, 2 * jj + 1):
                        otx, oty = ons[j]
                        base = (j % 2) * 256
                        psy = psy2[jj][:, base:base + C]
                        nc.tensor.matmul(psy, otx, wp_a,
                                         start=(j % 2 == 0), stop=False)
                        if not last:
                            nc.tensor.matmul(
                                psy, oty[64:P, :], wp_bb[64:P, :],
                                start=False, stop=False,
                                tile_position=(64, 0),
                                skip_group_check=True)
                        else:
                            nc.tensor.matmul(psy, oty, wp_b,
                                             start=False, stop=False,
                                             tile_position=(0, 0),
                                             skip_group_check=True)
                        nc.tensor.matmul(psy, ones1, bp_sb,
                                         start=False, stop=(j % 2 == 1))
                    for j in (2 * jj, 2 * jj + 1):
                        t0 = c * TCH + j * P
                        base = (j % 2) * 256
                        ysb = ysb_pool.tile([P, C], F32, name="ysb", tag="y")
                        (nc.scalar.copy if j % 2 == 0
                         else nc.vector.tensor_copy)(
                            ysb, psy2[jj][:, base:base + C])
                        nc.sync.dma_start(out[t0:t0 + P, :], ysb)

            return [lambda: (norm_t(0), norm_t(1)),
                    lambda: (norm_t(2), norm_t(3)),
                    proj_t]

        pairs = [(si, p3) for p3 in range(3) for si in range(NS)]
        npair = len(pairs)

        def emit_qkt_exp(c, pr):
            si, p3 = pairs[pr]
            ps = ps_tile("psst")
            for half in (0, 1):
                h = 2 * p3 + half
                kt, kr = ksrc(h)
                qt, qr = qsrc(h)
                nc.tensor.matmul(
                    ps[:, half * TCH:(half + 1) * TCH],
                    kt[kr:kr + HS, si * P:(si + 1) * P],
                    qt[qr:qr + HS, c * TCH:(c + 1) * TCH],
                    start=True, stop=True,
                    tile_position=(kr, 0), skip_group_check=True)
            dst = ptp[p3][si]
            if EXP_PAT[pr] == 0:
                nc.scalar.activation(dst, ps, Exp, scale=SCALE)
            else:
                nc.vector.tensor_scalar(dst[:, :].bitcast(I16), ps,
                                        AEXP, BEXP, MUL, ADD)

        pending_stagec = []
        pre_emitted = False
        for c in range(NCH):
            po2box = []

            def issue_pv(pr):
                po2 = po2box[0]
                si, p3 = pairs[pr]
                for half in (0, 1):
                    h = 2 * p3 + half
                    for j in range(NJ):
                        base = (j % 2) * 256
                        nc.tensor.matmul(
                            po2[j // 2][:, base + h * 33:base + (h + 1) * 33],
                            ptp[p3][si][:, half * TCH + j * P:
                                        half * TCH + (j + 1) * P],
                            va[si][:, h * 33:(h + 1) * 33],
                            start=(pr == 0 and half == 0 and j % 2 == 0),
                            stop=(pr == npair - 1 and half == 1
                                  and j % 2 == 1),
                            tile_position=(0, 0), skip_group_check=True)

            for pr, (si, p3) in enumerate(pairs):
                if c == 0:
                    for fn, args in sched.get(pr, []):
                        fn(*args)
                if pending_stagec and pr < len(pending_stagec):
                    pending_stagec[pr]()
                if pr == LAG - 1:
                    # the previous chunk's stage C is fully traced; now the
                    # PSUM bank tags can rotate to this chunk's accumulators
                    pending_stagec = []
                    po2box.append(
                        [po_pool.tile([P, TCH], F32, name=f"po{jj}",
                                      tag=f"po{jj}", bufs=1)
                         for jj in range(2)])
                if not (pre_emitted and pr < LAG):
                    emit_qkt_exp(c, pr)
                if pr >= LAG:
                    issue_pv(pr - LAG)
            # prime the next chunk's exp pipeline before this chunk's PV
            # tail so the boundary never drains the engines
            if c + 1 < NCH:
                for pr2 in range(LAG):
                    emit_qkt_exp(c + 1, pr2)
                pre_emitted = True
            for pr in range(npair - LAG, npair):
                issue_pv(pr)

            last = (c == NCH - 1)
            tasks = make_stagec(c, po2box[0], last)
            if last:
                for t in tasks:
                    t()
            else:
                pending_stagec = tasks

    nc.compile()
    return nc


def _get_nc():
    if "nc" not in _CACHE:
        _CACHE["nc"] = build_nc()
    return _CACHE["nc"]


def make_in_maps(x, Wq, Wk, Wv, Wproj, bproj):
    bf = ml_dtypes.bfloat16
    x = np.asarray(x, np.float32)
    pack = lambda w: np.ascontiguousarray(
        np.transpose(np.asarray(w, np.float32), (1, 0, 2)).reshape(C, H * HS))
    pq, pk = pack(Wq), pack(Wk)
    # col order: q0-3 | k0-3 | q4,q5 | k4,k5 (q_h/k_h at equal row bases)
    wqk = np.ascontiguousarray(np.concatenate(
        [pq[:, 0:128], pk[:, 0:128], pq[:, 128:192], pk[:, 128:192]],
        axis=1)).astype(bf)
    wv = pack(Wv).astype(bf)
    wp = np.ascontiguousarray(np.asarray(Wproj, np.float32)).astype(bf)
    bp = np.asarray(bproj, np.float32).reshape(1, C).astype(bf)
    ident = np.eye(P, dtype=np.float32).astype(bf)
    maps = []
    for i in range(B):
        xti = np.ascontiguousarray(x[i].T).astype(bf)
        maps.append({"xT": xti, "wqk": wqk, "wv": wv, "wp": wp, "bp": bp,
                     "ident": ident})
    return maps


def run(inputs, trace=False, **kw):
    nc = _get_nc()
    in_maps = make_in_maps(**inputs)
    res = run_bass_kernel_spmd(nc, in_maps, core_ids=list(range(B)),
                               trace=trace, **kw)
    y = np.stack([np.asarray(res.results[i]["out"], np.float32)
                  for i in range(B)], axis=0)
    return y, res


def kernel(**inputs):
    y, _ = run(inputs, trace=False)
    return y
